# revision 1
# baseline (speedup 1.0000x reference)
"""Trainium2 Bass kernel for causal multi-head attention (nn_MultiHeadAttention).

Full-input contract: kernel(**inputs) takes the complete tensors
(x [4,2048,1024] f32, Wq/Wk/Wv/Wp [1024,1024], bq/bk/bv/bp [1024]) and
returns the full output [4,2048,1024] f32.

Sharding: 8 cores = 4 batches x 2 head-groups (8 heads / 512 dims each).
Each core computes its head-group's attention output projected through its
row-slice of Wp; the host sums the two partial projections per batch and
adds (bv @ Wp + bp) (exact because softmax rows sum to 1, so the bv term
factors out of the attention).

Host prep (layout only): x is cast to bf16 and transposed to feature-major
xt [1024, 2048]; weight slices are cast to bf16. Device does pure compute:
  V  = x @ Wv  [2048, 512] stored with a per-head ones column (V_aug) so the
       attention matmul also produces the softmax denominator.
  QT = (Wq^T @ xT)/8 + bq/8, KT = Wk^T @ xT + bk   (both [512, 2048], bf16)
  Per head: scoresT[k, q] = KT_h-block^T @ QT_h (causal: only q >= k blocks),
  exp on ScalarE (no max-subtraction needed; |scores| < ~6 by construction),
  diagonal-block triangular mask, outT_aug[d+1, q] accumulated over k-tiles
  in two 1024-column halves (PSUM pressure), normalized via GpSimd
  partition-broadcast of the reciprocal ones-row, giving attnoutT [e, q].
  Final: out_partial[q, :] = attnoutT^T @ Wp_slice, all in bf16 matmuls with
  fp32 PSUM accumulation.
"""
import sys

sys.path.insert(0, "/opt/trn_rl_repo")

import numpy as np
import ml_dtypes

import concourse.bass as bass
import concourse.mybir as mybir
import concourse.tile as tile
from concourse import bacc
from concourse import bass_utils

N_CORES = 8
T = 2048          # tokens per batch
E = 1024          # model dim
D = 512           # head dims per core (8 heads x 64)
H = 8             # heads per core
DH = 64           # head dim
P = 128
FT = E // P       # 8 feature k-tiles
DT = D // P       # 4 local d-tiles
TT = T // P       # 16 token tiles
F32 = mybir.dt.float32
BF16 = mybir.dt.bfloat16
Alu = mybir.AluOpType
Act = mybir.ActivationFunctionType


def _build_program():
    nc = bacc.Bacc(
        "TRN2",
        target_bir_lowering=False,
        debug=False,
        enable_asserts=False,
        num_devices=N_CORES,
    )
    xt_d = nc.dram_tensor("xt", [E, T], BF16, kind="ExternalInput").ap()
    wq_d = nc.dram_tensor("wq", [E, D], BF16, kind="ExternalInput").ap()
    wk_d = nc.dram_tensor("wk", [E, D], BF16, kind="ExternalInput").ap()
    wv_d = nc.dram_tensor("wv", [E, D], BF16, kind="ExternalInput").ap()
    wp_d = nc.dram_tensor("wp", [D, E], BF16, kind="ExternalInput").ap()
    bq8_d = nc.dram_tensor("bq8", [P, DT], F32, kind="ExternalInput").ap()
    bk_d = nc.dram_tensor("bk", [P, DT], F32, kind="ExternalInput").ap()
    tri_d = nc.dram_tensor("tri", [P, P], BF16, kind="ExternalInput").ap()
    out_d = nc.dram_tensor("out", [T, E], F32, kind="ExternalOutput").ap()

    with tile.TileContext(nc) as tc:
        _kernel(tc, xt_d, wq_d, wk_d, wv_d, wp_d, bq8_d, bk_d, tri_d, out_d)
    nc.compile()
    return nc


def _kernel(tc, xt_d, wq_d, wk_d, wv_d, wp_d, bq8_d, bk_d, tri_d, out_d):
    nc = tc.nc
    from contextlib import ExitStack

    with ExitStack() as ctx:
        consts = ctx.enter_context(tc.tile_pool(name="consts", bufs=1))
        wpool = ctx.enter_context(tc.tile_pool(name="wpool", bufs=1))
        big = ctx.enter_context(tc.tile_pool(name="big", bufs=1))
        att = ctx.enter_context(tc.tile_pool(name="att", bufs=8))
        norm = ctx.enter_context(tc.tile_pool(name="norm", bufs=4))
        ostage = ctx.enter_context(tc.tile_pool(name="ostage", bufs=3))
        ps_sc = ctx.enter_context(tc.tile_pool(name="ps_sc", bufs=2, space="PSUM"))
        ps_mm = ctx.enter_context(tc.tile_pool(name="ps_mm", bufs=2, space="PSUM"))
        ps_out = ctx.enter_context(tc.tile_pool(name="ps_out", bufs=2, space="PSUM"))

        # ---- constants ----
        tri = consts.tile([P, P], BF16)
        nc.scalar.dma_start(out=tri, in_=tri_d)
        bq8 = consts.tile([P, DT], F32)
        nc.scalar.dma_start(out=bq8, in_=bq8_d)
        bk = consts.tile([P, DT], F32)
        nc.scalar.dma_start(out=bk, in_=bk_d)

        # ---- direct bf16 loads ----
        wq_b = wpool.tile([P, FT, D], BF16, tag="wq")
        wk_b = wpool.tile([P, FT, D], BF16, tag="wk")
        wv_b = wpool.tile([P, FT, D], BF16, tag="wv")
        wp_b = wpool.tile([P, DT, E], BF16, tag="wp")
        xT = big.tile([P, FT, T], BF16, tag="xT")
        xt_r = xt_d.rearrange("(ft p) t -> p ft t", p=P)
        wv_r = wv_d.rearrange("(ft p) d -> p ft d", p=P)
        # order: V(tt0)'s minimal gating set first (wv + x in ft halves),
        # then wq/wk, the rest of x, wp
        nc.sync.dma_start(out=wv_b[:, 0:4, :], in_=wv_r[:, 0:4, :])
        nc.sync.dma_start(out=xT[:, 0:4, 0:512], in_=xt_r[:, 0:4, 0:512])
        nc.sync.dma_start(out=wv_b[:, 4:8, :], in_=wv_r[:, 4:8, :])
        nc.sync.dma_start(out=xT[:, 4:8, 0:512], in_=xt_r[:, 4:8, 0:512])
        nc.sync.dma_start(out=wq_b, in_=wq_d.rearrange("(ft p) d -> p ft d", p=P))
        nc.sync.dma_start(out=wk_b, in_=wk_d.rearrange("(ft p) d -> p ft d", p=P))
        for tc_i in range(1, 4):
            nc.sync.dma_start(
                out=xT[:, :, tc_i * 512 : (tc_i + 1) * 512],
                in_=xt_r[:, :, tc_i * 512 : (tc_i + 1) * 512],
            )
        nc.sync.dma_start(out=wp_b, in_=wp_d.rearrange("(et p) e -> p et e", p=P))

        qt8 = big.tile([P, DT, T], BF16, tag="qt8")
        kt8 = big.tile([P, DT, T], BF16, tag="kt8")
        vaug = big.tile([P, TT, H * (DH + 1)], BF16, tag="vaug")
        aoutT = big.tile([P, DT, T], BF16, tag="aoutT")

        # ---- V with per-head ones column: [128, tt, 8*65] bf16 ----
        nc.vector.memset(
            vaug.rearrange("p tt (h x) -> p tt h x", x=DH + 1)[:, :, :, DH : DH + 1],
            1.0,
        )
        for tt in range(TT):
            va = vaug[:, tt, :].rearrange("p (h x) -> p h x", x=DH + 1)
            pv = ps_mm.tile([P, 512], F32, tag="mm")
            for ft in range(FT):
                nc.tensor.matmul(
                    pv,
                    lhsT=xT[:, ft, tt * P : (tt + 1) * P],
                    rhs=wv_b[:, ft, :],
                    start=(ft == 0),
                    stop=(ft == FT - 1),
                )
            nc.scalar.activation(
                out=va[:, :, 0:DH],
                in_=pv.rearrange("p (h d) -> p h d", d=DH),
                func=Act.Copy,
            )

        # ---- per d-tile: QT, KT, then 2 heads of attention ----
        pending = []
        for dt_i in range(DT):
            for dst, w_sb, bias, scale in (
                (qt8, wq_b, bq8, 0.125),
                (kt8, wk_b, bk, None),
            ):
                for c in range(4):
                    pq = ps_mm.tile([P, 512], F32, tag="mm")
                    for ft in range(FT):
                        nc.tensor.matmul(
                            pq,
                            lhsT=w_sb[:, ft, dt_i * P : (dt_i + 1) * P],
                            rhs=xT[:, ft, c * 512 : (c + 1) * 512],
                            start=(ft == 0),
                            stop=(ft == FT - 1),
                        )
                    dslice = dst[:, dt_i, c * 512 : (c + 1) * 512]
                    if scale is None:
                        nc.vector.tensor_scalar_add(
                            dslice, pq, bias[:, dt_i : dt_i + 1]
                        )
                    else:
                        nc.vector.tensor_scalar(
                            dslice, pq, scale, bias[:, dt_i : dt_i + 1],
                            op0=Alu.mult, op1=Alu.add,
                        )
            for h in (2 * dt_i, 2 * dt_i + 1):
                _head(tc, h, qt8, kt8, vaug, aoutT, tri, ps_sc, ps_out, att,
                      norm, pending)

        for fn, args in pending:
            fn(*args)

        # ---- output projection ----
        for qt in range(TT):
            ot = ostage.tile([P, E], F32, tag="ot")
            for oc in range(2):
                pp = ps_mm.tile([P, 512], F32, tag="mm")
                for et in range(DT):
                    nc.tensor.matmul(
                        pp,
                        lhsT=aoutT[:, et, qt * P : (qt + 1) * P],
                        rhs=wp_b[:, et, oc * 512 : (oc + 1) * 512],
                        start=(et == 0),
                        stop=(et == DT - 1),
                    )
                nc.scalar.activation(
                    out=ot[:, oc * 512 : (oc + 1) * 512], in_=pp, func=Act.Copy
                )
            nc.sync.dma_start(out=out_d[qt * P : (qt + 1) * P, :], in_=ot)


def _head(tc, h, qt8, kt8, vaug, aoutT, tri, ps_sc, ps_out, att, norm,
          pending):
    """Attention for one head, in two 1024-column q-halves.

    outT accumulates in per-512-column-chunk PSUM tiles; each chunk's
    accumulation (and its softmax denominator row) completes at
    kt == min(4c+3, 15), at which point it is normalized and released.
    """
    nc = tc.nc
    p0 = DH * (h % 2)
    dt_i = h // 2

    def normalize(c, outp):
        recip = norm.tile([1, 512], F32, tag="recip")
        nc.vector.reciprocal(recip, outp[DH : DH + 1, :])
        rb = norm.tile([DH, 512], F32, tag="rb")
        nc.gpsimd.partition_broadcast(rb, recip[0:1, :], channels=DH)
        nc.vector.tensor_tensor(
            aoutT[p0 : p0 + DH, dt_i, c * 512 : (c + 1) * 512],
            outp[0:DH, :],
            rb,
            op=Alu.mult,
        )

    for h2 in range(2):
        qbase = 1024 * h2
        outp = {}
        for c in (2 * h2, 2 * h2 + 1):
            outp[c] = ps_out.tile([DH + 1, 512], F32, tag="outT", name=f"outp{c}")

        def emit_out(kt, expT, q0, c_lo, base, outp=outp, h2=h2):
            # outT accumulation + chunk-completion normalize for one k-tile
            # (outp/h2 bound at definition: the pending queue outlives the half)
            va = vaug[:, kt, :].rearrange("p (h x) -> p h x", x=DH + 1)
            for c in range(c_lo, 2 * h2 + 2):
                cs = max(512 * c, q0)
                ce = 512 * (c + 1)
                nc.tensor.matmul(
                    outp[c][:, cs - 512 * c : ce - 512 * c],
                    lhsT=va[:, h, :],
                    rhs=expT[:, cs - base : ce - base],
                    start=(kt == 0),
                    stop=(kt == min(4 * c + 3, TT - 1)),
                )
            for c in range(c_lo, 2 * h2 + 2):
                if kt == min(4 * c + 3, TT - 1):
                    normalize(c, outp[c])

        # software-pipelined emission: outT for k-tile kt is emitted after
        # the scores/exp of kt+2 (the queue persists across halves and heads,
        # flushed by the caller before the projection)
        for kt in range(8 * (h2 + 1)):
            q0 = max(P * kt, qbase)
            c_lo = max(kt // 4, 2 * h2)
            base = 512 * c_lo  # tile column 0 <-> global q column `base`
            expT = att.tile([P, 1024], BF16, tag="expT")
            sp = ps_sc.tile([P, 1024], F32, tag="sc")
            for c in range(c_lo, 2 * h2 + 2):
                cs = max(512 * c, q0)
                ce = 512 * (c + 1)
                nc.tensor.matmul(
                    sp[:, cs - base : ce - base],
                    lhsT=kt8[p0 : p0 + DH, dt_i, kt * P : (kt + 1) * P],
                    rhs=qt8[p0 : p0 + DH, dt_i, cs:ce],
                    start=True,
                    stop=True,
                )
            nc.scalar.activation(
                out=expT[:, q0 - base : qbase + 1024 - base],
                in_=sp[:, q0 - base : qbase + 1024 - base],
                func=Act.Exp,
            )
            if kt // 8 == h2:
                # diagonal block: zero where q < k
                nc.vector.tensor_tensor(
                    expT[:, q0 - base : q0 - base + P],
                    expT[:, q0 - base : q0 - base + P],
                    tri,
                    op=Alu.mult,
                )
            pending.append((emit_out, (kt, expT, q0, c_lo, base)))
            if len(pending) > 4:
                fn, args = pending.pop(0)
                fn(*args)


_CACHED_NC = None


def _get_nc():
    global _CACHED_NC
    if _CACHED_NC is None:
        _CACHED_NC = _build_program()
    return _CACHED_NC


def make_in_maps(x, Wq, bq, Wk, bk, Wv, bv, Wp, bp):
    bf = ml_dtypes.bfloat16
    x = np.asarray(x, dtype=np.float32)
    tri = np.ascontiguousarray(np.triu(np.ones((P, P), np.float32)).astype(bf))
    in_maps = []
    wq_f = np.asarray(Wq, dtype=np.float32).astype(bf)
    wk_f = np.asarray(Wk, dtype=np.float32).astype(bf)
    wv_f = np.asarray(Wv, dtype=np.float32).astype(bf)
    wp_f = np.asarray(Wp, dtype=np.float32).astype(bf)
    for core in range(N_CORES):
        n, g = core // 2, core % 2
        sl = slice(g * D, (g + 1) * D)
        bq8 = (np.asarray(bq[sl], dtype=np.float32) / 8.0).reshape(DT, P).T
        bkc = np.asarray(bk[sl], dtype=np.float32).reshape(DT, P).T
        in_maps.append(
            {
                "xt": np.ascontiguousarray(x[n].T.astype(bf)),
                "wq": np.ascontiguousarray(wq_f[:, sl]),
                "wk": np.ascontiguousarray(wk_f[:, sl]),
                "wv": np.ascontiguousarray(wv_f[:, sl]),
                "wp": np.ascontiguousarray(wp_f[sl, :]),
                "bq8": np.ascontiguousarray(bq8),
                "bk": np.ascontiguousarray(bkc),
                "tri": tri,
            }
        )
    return in_maps


def assemble_output(results, Wv_b, Wp, bp, bv):
    corr = (np.asarray(bv, dtype=np.float32) @ np.asarray(Wp, dtype=np.float32)) + \
        np.asarray(bp, dtype=np.float32)
    out = np.empty((4, T, E), np.float32)
    for n in range(4):
        out[n] = results[2 * n]["out"] + results[2 * n + 1]["out"] + corr
    return out


def kernel(x, Wq, bq, Wk, bk, Wv, bv, Wp, bp):
    nc = _get_nc()
    in_maps = make_in_maps(x, Wq, bq, Wk, bk, Wv, bv, Wp, bp)
    res = bass_utils.run_bass_kernel_spmd(nc, in_maps, core_ids=list(range(N_CORES)))
    return assemble_output(res.results, Wv, Wp, bp, bv)



# revision 17
# speedup vs baseline: 1.0722x; 1.0722x over previous
"""Trainium2 Bass kernel for causal multi-head attention (nn_MultiHeadAttention).

Full-input contract: kernel(**inputs) takes the complete tensors
(x [4,2048,1024] f32, Wq/Wk/Wv/Wp [1024,1024], bq/bk/bv/bp [1024]) and
returns the full output [4,2048,1024] f32.

Sharding: 8 cores = 4 batches x 2 head-groups (8 heads / 512 dims each).
Each core computes its head-group's attention output projected through its
row-slice of Wp; the host sums the two partial projections per batch and
adds (bv @ Wp + bp) (exact because softmax rows sum to 1, so the bv term
factors out of the attention).

Device architecture (per core):
  V   = x @ Wv stored token-major with a per-head ones column (vaug), so the
        attn@V matmul also produces the softmax denominator.
  Q,K = projections accumulated in PSUM, then converted on DVE to fp8 e4m3
        (scaled by 16) into 9-slot zero-padded tiles [128, 9, 2048]: slot h
        holds head h's 64 dims on partitions 0:64 (upper partitions zero),
        slot 8 is all zeros.  A strided slice [:, h:9:(8-h), :] yields the
        [128, 2, N] operands DoubleRow requires (group 1 contributes zero).
  Scores: fp8 DoubleRow matmuls (0.5 cycles/row) into [128,1024] PSUM tiles,
        scoresT[k, q]; exp on ScalarE with scale 1/2048 (fp8 scale^2 * 1/8)
        into per-(half,kt) bf16 SBUF tiles; diagonal blocks masked on DVE.
  attn@V transposed: out[q, 65] accumulates over k-tiles with expT as the
        stationary operand and V (65 cols incl. ones) as moving -- 65 cycles
        per 128x128 block instead of 128.  Four q-blocks share one PSUM bank
        as a single accumulation group.  Normalization = DVE copy to SBUF +
        GpSimd normalize_recip (denominator is column 64).
  attnout [q, d] is PE-transposed (via identity) back to d-major aoutT for
        the output projection (bitcast view of the shared PSUM slot).
  Emission uses a credit scheduler: score matmuls are paced against a model
        of ScalarE exp completion, with V/QK/proj/transpose matmuls pulled
        from a filler deque in between so TensorE never stalls on exp.
  Two phases: all 8 heads' q-half [0,1024) first, then all q-half [1024,2048);
        the output projection for the low half runs as filler inside phase 2.
"""
import sys

sys.path.insert(0, "/opt/trn_rl_repo")

from collections import deque
from contextlib import ExitStack

import numpy as np
import ml_dtypes

import concourse.bass as bass
import concourse.mybir as mybir
import concourse.tile as tile
from concourse import bacc
from concourse import bass_utils

N_CORES = 8
T = 2048          # tokens per batch
E = 1024          # model dim
D = 512           # head dims per core (8 heads x 64)
H = 8             # heads per core
DH = 64           # head dim
P = 128
FT = E // P       # 8 feature k-tiles
DT = D // P       # 4 local d-tiles
TT = T // P       # 16 token tiles
F32 = mybir.dt.float32
BF16 = mybir.dt.bfloat16
FP8 = mybir.dt.float8e4
Alu = mybir.AluOpType
Act = mybir.ActivationFunctionType
DR = mybir.MatmulPerfMode.DoubleRow

USE_FP8 = True
QK_SCALE = 16.0                 # fp8 quantization scale for q and k
EXP_SCALE = 1.0 / (8.0 * QK_SCALE * QK_SCALE) if USE_FP8 else 1.0

# pacing model constants (units: PE cycles @2.4GHz; 1 cycle = 0.4167ns)
SEM_CYC = 260
EXP_OVERHEAD_CYC = 450
AV_MARGIN_CYC = 300


def _build_program():
    nc = bacc.Bacc(
        "TRN2",
        target_bir_lowering=False,
        debug=False,
        enable_asserts=False,
        num_devices=N_CORES,
    )
    xt_d = nc.dram_tensor("xt", [E, T], BF16, kind="ExternalInput").ap()
    wq_d = nc.dram_tensor("wq", [E, D], BF16, kind="ExternalInput").ap()
    wk_d = nc.dram_tensor("wk", [E, D], BF16, kind="ExternalInput").ap()
    wv_d = nc.dram_tensor("wv", [E, D], BF16, kind="ExternalInput").ap()
    wp_d = nc.dram_tensor("wp", [D, E], BF16, kind="ExternalInput").ap()
    bqs_d = nc.dram_tensor("bqs", [P, DT], F32, kind="ExternalInput").ap()
    bks_d = nc.dram_tensor("bks", [P, DT], F32, kind="ExternalInput").ap()
    tri_d = nc.dram_tensor("tri", [P, P], BF16, kind="ExternalInput").ap()
    z8_d = nc.dram_tensor("z8", [P, T], FP8, kind="ExternalInput").ap()
    eye_d = nc.dram_tensor("eye", [P, P], BF16, kind="ExternalInput").ap()
    out_d = nc.dram_tensor("out", [T, E], F32, kind="ExternalOutput").ap()
    out2_d = nc.dram_tensor("out2", [T // 2, E], F32, kind="ExternalOutput").ap()

    with tile.TileContext(nc) as tc:
        _kernel(tc, xt_d, wq_d, wk_d, wv_d, wp_d, bqs_d, bks_d, tri_d, eye_d,
                z8_d, out_d, out2_d)
    nc.compile()
    return nc


def _kernel(tc, xt_d, wq_d, wk_d, wv_d, wp_d, bqs_d, bks_d, tri_d, eye_d,
            z8_d, out_d, out2_d):
    nc = tc.nc

    with ExitStack() as ctx:
        consts = ctx.enter_context(tc.tile_pool(name="consts", bufs=1))
        wpool = ctx.enter_context(tc.tile_pool(name="wpool", bufs=1))
        big = ctx.enter_context(tc.tile_pool(name="big", bufs=1))
        q8pool = ctx.enter_context(tc.tile_pool(name="q8pool", bufs=1))
        att = ctx.enter_context(tc.tile_pool(name="att", bufs=1))
        aoutq = ctx.enter_context(tc.tile_pool(name="aoutq", bufs=1))
        stg = ctx.enter_context(tc.tile_pool(name="stg", bufs=4))
        ostage = ctx.enter_context(tc.tile_pool(name="ostage", bufs=3))
        ps_sc = ctx.enter_context(tc.tile_pool(name="ps_sc", bufs=2, space="PSUM"))
        ps_mm = ctx.enter_context(tc.tile_pool(name="ps_mm", bufs=2, space="PSUM"))
        ps_av = ctx.enter_context(tc.tile_pool(name="ps_av", bufs=2, space="PSUM"))

        # ---- eye first: it feeds the PE warmup ----
        eye = consts.tile([P, P], BF16)
        nc.scalar.dma_start(out=eye, in_=eye_d)

        # ---- weight/x loads, granular and ordered for early start ----
        wq_b = wpool.tile([P, FT, D], BF16, tag="wq")
        wk_b = wpool.tile([P, FT, D], BF16, tag="wk")
        wv_b = wpool.tile([P, FT, D], BF16, tag="wv")
        wp_b = wpool.tile([P, DT, E], BF16, tag="wp")
        xT = big.tile([P, FT, T], BF16, tag="xT")
        xt_r = xt_d.rearrange("(ft p) t -> p ft t", p=P)
        wq_r = wq_d.rearrange("(ft p) d -> p ft d", p=P)
        wk_r = wk_d.rearrange("(ft p) d -> p ft d", p=P)
        nc.sync.dma_start(out=wq_b[:, :, 0:P], in_=wq_r[:, :, 0:P])
        nc.sync.dma_start(out=xT[:, :, 0:512], in_=xt_r[:, :, 0:512])
        nc.sync.dma_start(out=wk_b[:, :, 0:P], in_=wk_r[:, :, 0:P])
        # remaining consts go behind the critical startup loads
        bqs = consts.tile([P, DT], F32)
        nc.scalar.dma_start(out=bqs, in_=bqs_d)
        bks = consts.tile([P, DT], F32)
        nc.scalar.dma_start(out=bks, in_=bks_d)
        tri = consts.tile([P, P], BF16)
        nc.scalar.dma_start(out=tri, in_=tri_d)
        nc.sync.dma_start(out=xT[:, :, 512:1024], in_=xt_r[:, :, 512:1024])
        nc.sync.dma_start(out=wq_b[:, :, P:D], in_=wq_r[:, :, P:D])
        nc.sync.dma_start(out=wk_b[:, :, P:D], in_=wk_r[:, :, P:D])
        nc.sync.dma_start(out=wv_b, in_=wv_d.rearrange("(ft p) d -> p ft d", p=P))
        nc.sync.dma_start(out=xT[:, :, 1024:1536], in_=xt_r[:, :, 1024:1536])
        nc.sync.dma_start(out=xT[:, :, 1536:2048], in_=xt_r[:, :, 1536:2048])
        nc.sync.dma_start(out=wp_b, in_=wp_d.rearrange("(et p) e -> p et e", p=P))

        # ---- persistent data tiles ----
        vaug = big.tile([P, TT, H * (DH + 1)], BF16, tag="vaug")
        aoutT = big.tile([P, DT, T], BF16, tag="aoutT")
        if USE_FP8:
            q8 = q8pool.tile([P, H + 1, T], FP8, tag="q8")
            k8 = q8pool.tile([P, H + 1, T], FP8, tag="k8")
        else:
            q8 = q8pool.tile([P, DT, T], BF16, tag="q8")
            k8 = q8pool.tile([P, DT, T], BF16, tag="k8")

        # zero-pad slots 2..7 of q8/k8 via DMA from the zeros buffer
        if USE_FP8:
            for t8 in (q8, k8):
                for slot in range(2, 8):
                    nc.sync.dma_start(out=t8[64:128, slot, :],
                                      in_=z8_d[64:128, :])

        # ones columns of vaug (denominator trick)
        nc.vector.memset(
            vaug.rearrange("p tt (h x) -> p tt h x", x=DH + 1)[:, :, :, DH:DH + 1],
            1.0,
        )
        # fp8 zero padding: slots 0..7 upper partitions + slot 8 entirely.
        # Slots {0,1,8} (phase-1 head 0 prerequisites) via fast Pool memsets;
        # slots 2..7 via DMA from a zeros buffer (idle DMA, frees Pool for
        # the normalizes it would otherwise gate).
        if USE_FP8:
            for t8 in (q8, k8):
                nc.gpsimd.memset(t8[64:128, 0, :], 0.0)
                nc.gpsimd.memset(t8[64:128, 1, :], 0.0)
                nc.gpsimd.memset(t8[:, 8, :], 0.0)

        # ---------------- emission state ----------------
        state = {"cyc": 0.0, "act": 0.0}
        exp_end = {}          # (h, half, kt) -> model cycle when exp done
        sc_hist = deque(maxlen=2)   # exp-done cycles of last 2 sc allocations
        filler = deque()      # (key, cycles, emit_fn)
        appended = set()      # every key ever placed in the deque
        emitted = set()
        tr_cnt = {}           # qt -> number of dt transposes emitted
        ot_tiles = {}
        aq_tiles = {}         # (dt, qt) -> aoutQ tile, created at first use

        def get_aq(dt_i, qt):
            if (dt_i, qt) not in aq_tiles:
                aq_tiles[(dt_i, qt)] = aoutq.tile(
                    [P, 2, DH], BF16, tag=f"aq{dt_i}_{qt}",
                    name=f"aq{dt_i}_{qt}",
                )
            return aq_tiles[(dt_i, qt)]

        def add_filler(key, cyc, fn):
            appended.add(key)
            filler.append((key, cyc, fn))

        def run_filler(credit):
            burned = 0.0
            while filler and burned < credit:
                key, cyc, fn = filler.popleft()
                fn()
                emitted.add(key)
                state["cyc"] += cyc
                burned += cyc

        def require(key):
            # pull from the deque until `key` has been emitted; keys never
            # appended are treated as satisfied (startup-emitted QK chunks)
            if key not in appended:
                return
            while filler and key not in emitted:
                k2, cyc, fn = filler.popleft()
                fn()
                emitted.add(k2)
                state["cyc"] += cyc

        def wait_model(target):
            # pull filler until the model says `target` (ACT-side) has passed
            while filler and state["cyc"] < target:
                key, cyc, fn = filler.popleft()
                fn()
                emitted.add(key)
                state["cyc"] += cyc

        # ---------------- unit emitters ----------------
        def emit_qk(dt_i, c, which):
            dst, w_sb, bias = (
                (q8, wq_b, bqs) if which == "q" else (k8, wk_b, bks)
            )
            pq = ps_mm.tile([P, 512], F32, tag="mm")
            for ft in range(FT):
                nc.tensor.matmul(
                    pq,
                    lhsT=w_sb[:, ft, dt_i * P:(dt_i + 1) * P],
                    rhs=xT[:, ft, c * 512:(c + 1) * 512],
                    start=(ft == 0),
                    stop=(ft == FT - 1),
                )
            cs = slice(c * 512, (c + 1) * 512)
            if USE_FP8:
                for hh in (0, 1):
                    h = 2 * dt_i + hh
                    nc.vector.tensor_scalar(
                        dst[0:DH, h, cs],
                        pq[hh * DH:(hh + 1) * DH, :],
                        QK_SCALE,
                        bias[hh * DH:(hh + 1) * DH, dt_i:dt_i + 1],
                        op0=Alu.mult,
                        op1=Alu.add,
                    )
            else:
                scale = 0.125 if which == "q" else 1.0
                nc.vector.tensor_scalar(
                    dst[:, dt_i, cs], pq, scale, bias[:, dt_i:dt_i + 1],
                    op0=Alu.mult, op1=Alu.add,
                )

        def emit_v(tt):
            va = vaug[:, tt, :].rearrange("p (h x) -> p h x", x=DH + 1)
            pv = ps_mm.tile([P, 512], F32, tag="mm")
            for ft in range(FT):
                nc.tensor.matmul(
                    pv,
                    lhsT=xT[:, ft, tt * P:(tt + 1) * P],
                    rhs=wv_b[:, ft, :],
                    start=(ft == 0),
                    stop=(ft == FT - 1),
                )
            nc.vector.tensor_copy(
                va[:, :, 0:DH], pv.rearrange("p (h d) -> p h d", d=DH)
            )

        def emit_tr(dt_i, a):
            # transpose aoutQ(dt, qt) [q, d] -> aoutT [d, q] for 4 qt
            for qi in range(4):
                qt = 4 * a + qi
                aq = get_aq(dt_i, qt)
                slot = ps_mm.tile([P, 512], F32, tag="mm")
                ptr = slot.bitcast(BF16)[:, 0:P]
                nc.tensor.transpose(ptr, aq.rearrange("p a b -> p (a b)"), eye)
                nc.vector.tensor_copy(
                    aoutT[:, dt_i, qt * P:(qt + 1) * P], ptr
                )
                tr_cnt[qt] = tr_cnt.get(qt, 0) + 1
                # qt < 8: full projection once all 4 dt transposed.
                # qt >= 8: partial over et 0,1 (-> out2, host-summed) as soon
                # as dt0/dt1 are transposed, remainder et 2,3 at the tail --
                # keeps the post-h7 tensor tail to half a projection.
                if qt < 8 and tr_cnt[qt] == DT:
                    for oc in (0, 1):
                        add_filler(f"proj{qt}_{oc}", 2048.0,
                                   lambda qt=qt, oc=oc: emit_proj(qt, oc, "full"))
                elif qt >= 8 and tr_cnt[qt] == 2:
                    for oc in (0, 1):
                        add_filler(f"proja{qt}_{oc}", 1024.0,
                                   lambda qt=qt, oc=oc: emit_proj(qt, oc, "A"))
                elif qt >= 8 and tr_cnt[qt] == DT:
                    for oc in (0, 1):
                        add_filler(f"projb{qt}_{oc}", 1024.0,
                                   lambda qt=qt, oc=oc: emit_proj(qt, oc, "B"))

        def emit_proj(qt, oc, part):
            ets = {"full": range(DT), "A": (0, 1), "B": (2, 3)}[part]
            pp = ps_mm.tile([P, 512], F32, tag="mm")
            for i, et in enumerate(ets):
                nc.tensor.matmul(
                    pp,
                    lhsT=aoutT[:, et, qt * P:(qt + 1) * P],
                    rhs=wp_b[:, et, oc * 512:(oc + 1) * 512],
                    start=(i == 0),
                    stop=(i == len(list(ets)) - 1),
                )
            if part == "A":
                ot2 = ostage.tile([P, 512], F32, tag="ot2", name=f"ot2_{qt}_{oc}")
                nc.vector.tensor_copy(ot2, pp)
                nc.sync.dma_start(
                    out=out2_d[(qt - 8) * P:(qt - 7) * P, oc * 512:(oc + 1) * 512],
                    in_=ot2,
                )
                return
            if oc == 0:
                ot_tiles[qt] = ostage.tile([P, E], F32, tag="ot",
                                           name=f"ot{qt}")
            ot = ot_tiles[qt]
            nc.vector.tensor_copy(ot[:, oc * 512:(oc + 1) * 512], pp)
            if oc == 1:
                nc.sync.dma_start(out=out_d[qt * P:(qt + 1) * P, :], in_=ot)

        # ---------------- attention streams ----------------
        def scores_mm(sp, h, kt, s, e, base):
            """one scores piece: q columns [s,e) (global), k-tile kt"""
            dt_i, hh = h // 2, h % 2
            if USE_FP8:
                nc.tensor.matmul(
                    sp[:, s - base:e - base],
                    lhsT=k8[:, h:H + 1:H - h, kt * P:(kt + 1) * P],
                    rhs=q8[:, h:H + 1:H - h, s:e],
                    start=True, stop=True, perf_mode=DR,
                )
                return (e - s) * 0.5
            p0 = DH * hh
            nc.tensor.matmul(
                sp[:, s - base:e - base],
                lhsT=k8[p0:p0 + DH, dt_i, kt * P:(kt + 1) * P],
                rhs=q8[p0:p0 + DH, dt_i, s:e],
                start=True, stop=True,
            )
            return float(e - s)

        exps_of = {}

        def emit_sc_range(h, half, kt_lo, kt_hi):
            dt_i = h // 2
            base = half * 1024
            c_lo = 2 * half          # q chunks for this half: c_lo, c_lo+1
            if kt_lo == 0:
                exps_of[(h, half)] = []
            exps = exps_of[(h, half)]
            for kt in range(kt_lo, kt_hi):
                # just-in-time operand requires, spread across the kt loop
                if kt == 0:
                    require(f"qk{dt_i}_{c_lo}q")
                    require(f"qk{dt_i}_{c_lo}k" if half == 0
                            else f"qk{dt_i}_0k")
                    require(f"qk{dt_i}_{c_lo + 1}q")
                if half == 0 and kt == 4:
                    require(f"qk{dt_i}_1k")
                if half == 1:
                    if kt == 4:
                        require(f"qk{dt_i}_1k")
                    elif kt == 8:
                        require(f"qk{dt_i}_2k")
                    elif kt == 12:
                        require(f"qk{dt_i}_3k")
                q0 = max(kt * P, base)
                span = base + 1024 - q0
                # pace: sc pool has 2 bufs; wait for exp of 2 allocations ago
                if len(sc_hist) == 2:
                    wait_model(sc_hist[0] - 1200)
                sp = ps_sc.tile([P, 1024], F32, tag="sc")
                et = att.tile([P, span], BF16, tag=f"e{half}_{kt}",
                              name=f"e{half}_{kt}")
                bsplit = base + 512
                if h == 0 and half == 0 and kt < 2:
                    # startup: exp each 512-piece as soon as its chunk of
                    # q8/k8 exists, so ScalarE starts ~3us earlier
                    state["cyc"] += scores_mm(sp, h, kt, q0, bsplit, base)
                    nc.scalar.activation(
                        out=et[:, 0:bsplit - q0], in_=sp[:, q0 - base:512],
                        func=Act.Exp, scale=EXP_SCALE,
                    )
                    nc.vector.tensor_tensor(
                        et[:, 0:P], et[:, 0:P], tri, op=Alu.mult
                    )
                    t0 = max(state["act"], state["cyc"] + SEM_CYC)
                    state["act"] = t0 + (bsplit - q0) * 2.0 + EXP_OVERHEAD_CYC
                    require("qk0_1q")
                    require("qk0_1k")
                    state["cyc"] += scores_mm(sp, h, kt, bsplit, base + 1024,
                                              base)
                    nc.scalar.activation(
                        out=et[:, bsplit - q0:span],
                        in_=sp[:, 512:1024], func=Act.Exp, scale=EXP_SCALE,
                    )
                    t1 = max(state["act"], state["cyc"] + SEM_CYC)
                    done = t1 + 512 * 2.0 + EXP_OVERHEAD_CYC
                else:
                    pieces = ([(q0, base + 1024)] if q0 >= bsplit
                              else [(q0, bsplit), (bsplit, base + 1024)])
                    for (s, e) in pieces:
                        state["cyc"] += scores_mm(sp, h, kt, s, e, base)
                    t0 = max(state["act"], state["cyc"] + SEM_CYC)
                    done = t0 + span * 2.0 + EXP_OVERHEAD_CYC
                    nc.scalar.activation(
                        out=et, in_=sp[:, q0 - base:1024], func=Act.Exp,
                        scale=EXP_SCALE,
                    )
                    if kt * P >= base:  # diagonal block is first 128 cols
                        nc.vector.tensor_tensor(
                            et[:, 0:P], et[:, 0:P], tri, op=Alu.mult
                        )
                state["act"] = done
                exp_end[(h, half, kt)] = done
                sc_hist.append(done)
                exps.append(et)

        def emit_av_group(h, half, a):
            dt_i, hh = h // 2, h % 2
            base = half * 1024
            ktmax = 8 * (half + 1)
            exps = exps_of[(h, half)]
            if True:
                kneed = min(4 * a + 3, ktmax - 1)
                wait_model(exp_end[(h, half, kneed)] - 600)
                pav = ps_av.tile([P, 4, DH + 1], F32, tag="av")
                # kt-outer so expT tags are last-read in production order,
                # freeing them for the next head's exp as early as possible
                for kt in range(4 * a + 4):
                    require(f"v{kt}")
                    q0 = max(kt * P, base)
                    et = exps[kt]
                    for qi in range(4):
                        qt = 4 * a + qi
                        if qt < kt:
                            continue
                        nc.tensor.matmul(
                            pav[:, qi, :],
                            lhsT=et[:, qt * P - q0:qt * P - q0 + P],
                            rhs=vaug[:, kt, :].rearrange(
                                "p (hx x) -> p hx x", x=DH + 1)[:, h, :],
                            start=(kt == 0 and qi == 0),
                            stop=(kt == 4 * a + 3 and qi == 3),
                        )
                        state["cyc"] += DH + 1
                # normalize: DVE copy to SBUF stage, then gpsimd recip-normalize
                sg = stg.tile([P, 4, DH + 1], F32, tag="sg")
                nc.vector.tensor_copy(sg, pav)
                for qi in range(4):
                    qt = 4 * a + qi
                    aq = get_aq(dt_i, qt)
                    nc.gpsimd.normalize_recip(
                        aq[:, hh, :], sg[:, qi, 0:DH], sg[:, qi, DH:DH + 1]
                    )
                if hh == 1:
                    add_filler(f"tr{dt_i}_{a}", 512.0,
                               lambda dt_i=dt_i, a=a: emit_tr(dt_i, a))

        # ---------------- schedule ----------------
        # PE p-state warmup: harmless matmuls on the (early-loaded) identity
        # keep the PE busy through its frequency ramp while x/weights stream
        # in, so the first real matmuls run at full clock.
        for _ in range(24):
            wslot = ps_mm.tile([P, 512], F32, tag="mm")
            nc.tensor.matmul(wslot[:, 0:P], lhsT=eye, rhs=eye,
                             start=True, stop=True)

        # startup: QK for dt0 chunk 0 only; chunk 1 sits at the deque front
        # and is pulled between the first split scores pieces
        for which in ("q", "k"):
            emit_qk(0, 0, which)
            state["cyc"] += 4096

        # filler deque: V tiles first (needed by phase-1 av), then remaining
        # QK chunks ordered by first use, then V 8..15, then QK c2/c3.
        for which in ("q", "k"):
            add_filler(f"qk0_1{which}", 4096.0,
                       lambda which=which: emit_qk(0, 1, which))
        for tt in range(8):
            add_filler(f"v{tt}", 4096.0, lambda tt=tt: emit_v(tt))
        for c in (0, 1):
            for which in ("q", "k"):
                add_filler(f"qk1_{c}{which}", 4096.0,
                           lambda c=c, which=which: emit_qk(1, c, which))
        def qk_unit(dt_i, c):
            for which in ("q", "k"):
                add_filler(f"qk{dt_i}_{c}{which}", 4096.0,
                           lambda dt_i=dt_i, c=c, which=which:
                           emit_qk(dt_i, c, which))

        qk_unit(0, 2)
        qk_unit(0, 3)
        for tt in range(8, 12):
            add_filler(f"v{tt}", 4096.0, lambda tt=tt: emit_v(tt))
        for tt in range(12, 16):
            add_filler(f"v{tt}", 4096.0, lambda tt=tt: emit_v(tt))
        qk_unit(2, 0)
        qk_unit(2, 1)
        qk_unit(1, 2)
        qk_unit(1, 3)
        qk_unit(3, 0)
        qk_unit(3, 1)
        qk_unit(2, 2)
        qk_unit(2, 3)
        qk_unit(3, 2)
        qk_unit(3, 3)

        # software-pipelined stream.  The sc-pool rotation makes sc(h,kt2)
        # depend (via exp(h,kt0)) on the expT tags still held by the previous
        # same-half head's last av group, so that group is emitted between
        # sc(h, kt0-1) and sc(h, kt2+): tags release just in time and ScalarE
        # runs continuously.  Low-q halves (tensor-heavy: they gate V/QK
        # filler) and high-q halves (exp-heavy) are interleaved so tensor and
        # ScalarE load stay balanced through the whole run.
        stream = [(0, 0), (1, 0), (2, 0), (0, 1), (3, 0), (1, 1),
                  (4, 0), (2, 1), (5, 0), (3, 1), (6, 0), (4, 1),
                  (7, 0), (5, 1), (6, 1), (7, 1)]
        prev_last = {0: None, 1: None}
        for (h, half) in stream:
            emit_sc_range(h, half, 0, 2)
            if prev_last[half] is not None:
                emit_av_group(*prev_last[half])
            emit_sc_range(h, half, 2, 8 * (half + 1))
            g0, g1 = (0, 1) if half == 0 else (2, 3)
            emit_av_group(h, half, g0)
            prev_last[half] = (h, half, g1)
        emit_av_group(*prev_last[0])
        emit_av_group(*prev_last[1])
        # tail: remaining filler (transposes + proj for the high half)
        run_filler(float("inf"))


_CACHED_NC = None


def _get_nc():
    global _CACHED_NC
    if _CACHED_NC is None:
        _CACHED_NC = _build_program()
    return _CACHED_NC


def make_in_maps(x, Wq, bq, Wk, bk, Wv, bv, Wp, bp):
    bf = ml_dtypes.bfloat16
    x = np.asarray(x, dtype=np.float32)
    tri = np.ascontiguousarray(np.triu(np.ones((P, P), np.float32)).astype(bf))
    z8 = np.zeros((P, T), dtype=ml_dtypes.float8_e4m3)
    eye = np.eye(P, dtype=np.float32).astype(bf)
    bias_scale = QK_SCALE if USE_FP8 else 1.0
    q_bias_scale = QK_SCALE if USE_FP8 else 0.125
    in_maps = []
    wq_f = np.asarray(Wq, dtype=np.float32).astype(bf)
    wk_f = np.asarray(Wk, dtype=np.float32).astype(bf)
    wv_f = np.asarray(Wv, dtype=np.float32).astype(bf)
    wp_f = np.asarray(Wp, dtype=np.float32).astype(bf)
    for core in range(N_CORES):
        n, g = core // 2, core % 2
        sl = slice(g * D, (g + 1) * D)
        bqs = (np.asarray(bq[sl], dtype=np.float32) * q_bias_scale)
        bqs = bqs.reshape(DT, P).T
        bks = (np.asarray(bk[sl], dtype=np.float32) * bias_scale)
        bks = bks.reshape(DT, P).T
        in_maps.append(
            {
                "xt": np.ascontiguousarray(x[n].T.astype(bf)),
                "wq": np.ascontiguousarray(wq_f[:, sl]),
                "wk": np.ascontiguousarray(wk_f[:, sl]),
                "wv": np.ascontiguousarray(wv_f[:, sl]),
                "wp": np.ascontiguousarray(wp_f[sl, :]),
                "bqs": np.ascontiguousarray(bqs),
                "bks": np.ascontiguousarray(bks),
                "tri": tri,
                "eye": eye,
                "z8": z8,
            }
        )
    return in_maps


def assemble_output(results, Wp, bp, bv):
    corr = (np.asarray(bv, dtype=np.float32) @ np.asarray(Wp, dtype=np.float32)) + \
        np.asarray(bp, dtype=np.float32)
    out = np.empty((4, T, E), np.float32)
    for n in range(4):
        out[n] = results[2 * n]["out"] + results[2 * n + 1]["out"] + corr
        out[n][T // 2:] += results[2 * n]["out2"] + results[2 * n + 1]["out2"]
    return out


def kernel(x, Wq, bq, Wk, bk, Wv, bv, Wp, bp):
    nc = _get_nc()
    in_maps = make_in_maps(x, Wq, bq, Wk, bk, Wv, bv, Wp, bp)
    res = bass_utils.run_bass_kernel_spmd(nc, in_maps, core_ids=list(range(N_CORES)))
    return assemble_output(res.results, Wp, bp, bv)
